# revision 56
# baseline (speedup 1.0000x reference)
"""3-layer GAT on 8 Trainium2 NeuronCores — v3.

Strategy (edge-parallel, dst-sharded).  Revised after TimelineSim
profiling (per-instruction SWDGE overhead on Pool + f32 gather traffic
dominated v1) and hardware probing (multi-index `indirect_dma_start`
offsets silently degrade to one contiguous block per partition, so the
batched gathers use the `dma_gather` ISA op, which was probed to work,
including strided rows and column-sliced tables).

  - Nodes are relabeled into degree-balanced 128-node bins; each core
    owns bpc bins (dst-sharded).  A "super" == one dst tile.
  - Per layer, the node phase computes packed rows
    [h | a_src | a_dst | pad] (bf16, 640 cols for layers 0/1, 128 cols
    for layer 2 -- widths forced by dma_gather's 256B granularity); an
    AllGather replicates them into a [npad, width] table.
  - dma_gather indices are int16, so the table is addressed in two
    halves (rows < 32768 and the rest).  The host orders each tile's
    edges low-half first (chunks 0..K1-1), high-half after, padding
    each half to chunk boundaries; every super then does exactly two
    batched row gathers.
  - Per-edge a_dst comes from a per-chunk one-hot matmul (pt) against
    the dst tile's a_dst rows; a_src rides inside the gathered row.
  - The one-hot scatter (pm) and lookup (pt) matrices are pure graph
    structure: host-built, shipped as fp8e4m3 (0/1 exact; probed OK as
    lhsT against bf16 rhs), one combined load per super.
  - exp(leaky_relu(a_src+a_dst)) on DVE/ACT (bf16), sc = h*alpha on
    DVE (optionally part on Pool), then per chunk one scatter-matmul
    (messages) + one denominator matmul accumulate in PSUM; flush
    normalizes, adds bias, relu, and writes transposed activations
    (PE transpose + one ACT copy + one strided DMA).
  - Pad edges point at row 0 with all-zero one-hot columns.

The module builds and compiles the Bass program on first call (keyed by
input shapes) and reuses it afterwards.
"""

import sys

try:
    import concourse  # noqa: F401  (provided via PYTHONPATH on axon hosts)
except ImportError:
    sys.path.insert(0, "/opt/trn_rl_repo")

import heapq

import numpy as np
from ml_dtypes import bfloat16, float8_e4m3

import concourse.bacc as bacc
import concourse.mybir as mybir
import concourse.tile as tile

P = 128
NCORES = 8
NH = 8          # heads (layers 0/1)
HC = 64         # channels per head
HD = NH * HC    # 512
D0 = 128        # input feature dim
OUTC = 40       # final classes
CO = 64         # padded layer-2 width
TW = 768        # table row bytes, layers 0/1 (fp8 tensor):
                #   h fp8 [0:512] | asrc f32 [512:544] | adst f32 [544:576] | pad
TW2 = 128       # table row width, layer 2:   h2(64)|asrc2(1)|adst2(1)|pad
NEG = 0.2       # leaky relu slope
LO = 32768      # int16 index limit (table split point)

# number of trailing chunks per super whose alpha*h multiply runs on the
# Pool engine instead of DVE (load balancing knob; 0 = all on DVE)
POOL_CHUNKS = 4

f32 = mybir.dt.float32
bf16 = mybir.dt.bfloat16
fp8 = mybir.dt.float8e4
i16 = mybir.dt.int16
AF = mybir.ActivationFunctionType
ALU = mybir.AluOpType


# ----------------------------------------------------------------- host prep

def _balance_bins(deg, nbins):
    """Assign each node to a 128-slot bin, balancing summed in-degree."""
    n = deg.shape[0]
    order = np.argsort(-deg, kind="stable")
    bin_of = np.empty(n, np.int32)
    slot_of = np.empty(n, np.int32)
    counts = np.zeros(nbins, np.int32)
    loads = np.zeros(nbins, np.int64)
    heap = [(0, b) for b in range(nbins)]
    heapq.heapify(heap)
    for node in order:
        while True:
            _, b = heapq.heappop(heap)
            if counts[b] < P:
                break
        bin_of[node] = b
        slot_of[node] = counts[b]
        counts[b] += 1
        loads[b] += deg[node]
        if counts[b] < P:
            heapq.heappush(heap, (int(loads[b]), b))
    return bin_of, slot_of, loads


def _wrap16(flat):
    """Lay a flat int16 index vector in dma_gather's wrapped order:
    position i -> [i % 16, i // 16], replicated across the 8 gpsimd
    core groups (partitions 16..127)."""
    n = flat.shape[0]
    lay = np.zeros((P, n // 16), np.int16)
    lay[:16, :] = flat.reshape(n // 16, 16).T
    for grp in range(1, 8):
        lay[grp * 16:(grp + 1) * 16, :] = lay[:16, :]
    return lay


def _prep(edge_index, n_nodes):
    src = np.asarray(edge_index[0], dtype=np.int64)
    dst = np.asarray(edge_index[1], dtype=np.int64)
    loop = np.arange(n_nodes, dtype=np.int64)
    src = np.concatenate([src, loop])
    dst = np.concatenate([dst, loop])

    deg = np.bincount(dst, minlength=n_nodes)
    bpc = -(-n_nodes // (P * NCORES))          # bins per core
    nbins = bpc * NCORES
    npad = nbins * P
    nloc = bpc * P

    bin_of, slot_of, loads = _balance_bins(deg, nbins)
    new_id = bin_of.astype(np.int64) * P + slot_of

    e_src = new_id[src]
    e_dst = new_id[dst]
    e_bin = e_dst >> 7
    e_slot = e_dst & 127

    order_e = np.argsort(e_bin, kind="stable")
    starts = np.zeros(nbins + 1, np.int64)
    starts[1:] = np.cumsum(np.bincount(e_bin, minlength=nbins))

    # per-bin low/high (by source id) counts -> uniform chunk split.
    # The split point S is searched to minimize total chunks per tile,
    # subject to both table halves having < 32768 rows (int16 indices).
    smin = max(1, npad - LO)
    best = None
    for S in range(smin, LO + 1, 256):
        is_lo = (e_src < S)
        nlo = np.bincount(e_bin[is_lo], minlength=nbins)
        nhi = np.bincount(e_bin[~is_lo], minlength=nbins)
        k1 = int(-(-nlo.max() // P))
        k2 = int(-(-nhi.max() // P))
        if best is None or k1 + k2 < best[0]:
            best = (k1 + k2, k1, S)
    G, K1, SPL = best

    idx_arr = np.zeros((NCORES, bpc, P, G * 8), np.int16)
    pmpt = np.zeros((NCORES, bpc, P, 2 * G * P), float8_e4m3)
    one8 = float8_e4m3(1.0)

    for b in range(nbins):
        c, t = divmod(b, bpc)
        es = order_e[starts[b]:starts[b + 1]]
        srcs = e_src[es]
        slots = e_slot[es].astype(np.int64)
        lo_m = srcs < SPL
        for half, (mask, base, k0, kcnt) in enumerate(
                [(lo_m, 0, 0, K1), (~lo_m, SPL, K1, G - K1)]):
            sv = (srcs[mask] - base).astype(np.int16)
            sl = slots[mask]
            ncap = kcnt * P
            flat = np.zeros(ncap, np.int16)
            flat[:sv.shape[0]] = sv
            idx_arr[c, t, :, k0 * 8:(k0 + kcnt) * 8] = _wrap16(flat)
            i = np.arange(sv.shape[0])
            p_of = i & 127
            g_of = k0 + (i >> 7)
            pmpt[c, t, p_of, g_of * P + sl] = one8                 # pm
            pmpt[c, t, sl, G * P + g_of * P + p_of] = one8         # pt

    per_core = []
    for c in range(NCORES):
        per_core.append({
            "idx": np.ascontiguousarray(idx_arr[c]).reshape(-1),
            "pmpt": np.ascontiguousarray(pmpt[c]).reshape(-1),
        })

    dims = dict(n=n_nodes, bpc=bpc, nbins=nbins, npad=npad, nloc=nloc,
                G=G, K1=K1, SPL=SPL)
    return dims, per_core, new_id


def _block_diag_a(att_s, att_d):
    """[NH,HC]x2 -> [HD, 2*NH] block matrix for a = h @ A."""
    a = np.zeros((HD, 2 * NH), np.float32)
    r = np.arange(HD)
    h = r >> 6
    c = r & 63
    a[r, h] = att_s[h, c]
    a[r, NH + h] = att_d[h, c]
    return a


# ------------------------------------------------------------- device build

def _build(dims, single=False, dbg=False):
    npad, nloc, bpc = dims["npad"], dims["nloc"], dims["bpc"]
    G, K1, SPL = dims["G"], dims["K1"], dims["SPL"]
    K2 = G - K1

    nc = bacc.Bacc("TRN2", target_bir_lowering=False, debug=False,
                   enable_asserts=True,
                   num_devices=1 if single else NCORES)

    # inputs
    xt_ap = nc.dram_tensor("xt", [D0, nloc], bf16, kind="ExternalInput").ap()
    idx_ap = nc.dram_tensor("idx", [bpc * P * G * 8], i16,
                            kind="ExternalInput").ap()
    pmpt_ap = nc.dram_tensor("pmpt", [bpc * P * 2 * G * P], fp8,
                             kind="ExternalInput").ap()
    w0h_ap = nc.dram_tensor("w0h", [D0, HD], bf16, kind="ExternalInput").ap()
    w0a_ap = nc.dram_tensor("w0a", [D0, 2 * NH], bf16,
                            kind="ExternalInput").ap()
    w1_ap = nc.dram_tensor("w1", [HD, HD], bf16, kind="ExternalInput").ap()
    w1a_ap = nc.dram_tensor("w1a", [HD, 2 * NH], bf16,
                            kind="ExternalInput").ap()
    w2e_ap = nc.dram_tensor("w2e", [HD, TW2], bf16,
                            kind="ExternalInput").ap()
    b0_ap = nc.dram_tensor("b0r", [1, HD], f32, kind="ExternalInput").ap()
    b1_ap = nc.dram_tensor("b1r", [1, HD], f32, kind="ExternalInput").ap()
    b2_ap = nc.dram_tensor("b2r", [1, CO], f32, kind="ExternalInput").ap()
    idb_ap = nc.dram_tensor("identb", [P, P], bf16, kind="ExternalInput").ap()
    out_ap = nc.dram_tensor("out", [nloc, OUTC], f32, kind="ExternalOutput").ap()

    with tile.TileContext(nc) as tc:
        with tc.tile_pool(name="const", bufs=1) as cp, \
             tc.tile_pool(name="work", bufs=4) as sb, \
             tc.tile_pool(name="big", bufs=2) as bb, \
             tc.tile_pool(name="psum", bufs=2, space="PSUM") as ps, \
             tc.tile_pool(name="dram", bufs=1, space="DRAM") as dp:

            # ---------- persistent constants in SBUF
            identb_t = cp.tile([P, P], bf16)
            nc.sync.dma_start(identb_t[:], idb_ap[:])
            ones_t = cp.tile([1, P], f32)
            nc.gpsimd.memset(ones_t[:], 1.0)

            w0h_t = cp.tile([P, HD], bf16)
            nc.sync.dma_start(w0h_t[:], w0h_ap[:])
            w0a_t = cp.tile([P, 2 * NH], bf16)
            nc.sync.dma_start(w0a_t[:], w0a_ap[:])
            w1_t = [cp.tile([P, HD], bf16, name=f"w1c{k}", tag=f"w1_{k}")
                    for k in range(4)]
            w1a_t = [cp.tile([P, 2 * NH], bf16, name=f"w1ac{k}", tag=f"w1a_{k}")
                     for k in range(4)]
            w2e_t = [cp.tile([P, TW2], bf16, name=f"w2ec{k}", tag=f"w2e_{k}")
                     for k in range(4)]
            for k in range(4):
                nc.sync.dma_start(w1_t[k][:], w1_ap[k * P:(k + 1) * P, :])
                nc.sync.dma_start(w1a_t[k][:], w1a_ap[k * P:(k + 1) * P, :])
                nc.sync.dma_start(w2e_t[k][:], w2e_ap[k * P:(k + 1) * P, :])

            def bias_tile(b_ap, width, tag):
                row = sb.tile([1, width], f32, tag="brow")
                nc.sync.dma_start(row[:], b_ap[:])
                bps = ps.tile([P, width], f32, tag="agg", bufs=2)
                nc.tensor.matmul(out=bps[:], lhsT=ones_t[:], rhs=row[:],
                                 start=True, stop=True)
                bt = cp.tile([P, width], f32, name=tag, tag=tag)
                nc.scalar.activation(bt[:], bps[:], AF.Copy)
                return bt

            b0_t = bias_tile(b0_ap, HD, "b0t")
            b1_t = bias_tile(b1_ap, HD, "b1t")
            b2_t = bias_tile(b2_ap, CO, "b2t")

            # ---------- DRAM scratch
            ag0 = dp.tile([nloc, TW], fp8)
            ag1 = dp.tile([nloc, TW], fp8)
            ag2 = dp.tile([nloc, TW2], bf16)
            hcat0 = dp.tile([npad, TW], fp8)
            hcat1 = dp.tile([npad, TW], fp8)
            hcat2 = dp.tile([npad, TW2], bf16)
            actt1 = dp.tile([nloc, HD], bf16)
            actt2 = dp.tile([nloc, HD], bf16)

            # ---------- node phases
            def node0():
                for t in range(bpc):
                    xt_t = sb.tile([P, P], bf16, tag="lhs")
                    nc.scalar.dma_start(xt_t[:], xt_ap[:, t * P:(t + 1) * P])
                    h_ps = ps.tile([P, HD], f32, tag="agg", bufs=2)
                    a_ps = ps.tile([P, 2 * NH], f32, tag="den", bufs=2)
                    nc.tensor.matmul(out=h_ps[:], lhsT=xt_t[:], rhs=w0h_t[:],
                                     start=True, stop=True)
                    nc.tensor.matmul(out=a_ps[:], lhsT=xt_t[:], rhs=w0a_t[:],
                                     start=True, stop=True)
                    cat = sb.tile([P, 576], fp8, tag="cat")
                    nc.scalar.activation(cat[:, :HD], h_ps[:], AF.Copy)
                    nc.vector.tensor_copy(cat[:, HD:576].bitcast(f32),
                                          a_ps[:])
                    nc.scalar.dma_start(ag0[t * P:(t + 1) * P, 0:576],
                                        cat[:])

            lhs_pend = {}

            def node_lhs_load(actt, t):
                lhs = sb.tile([P, 4 * P], bf16, tag="lhs")
                nc.scalar.dma_start(lhs[:], actt[t * P:(t + 1) * P, :])
                lhs_pend[t] = lhs

            def node_mid_t(actt, ag, t):
                if t not in lhs_pend:
                    node_lhs_load(actt, t)
                lhs = lhs_pend.pop(t)
                h_ps = ps.tile([P, HD], f32, tag="agg", bufs=2)
                a_ps = ps.tile([P, 2 * NH], f32, tag="den", bufs=2)
                for k in range(4):
                    nc.tensor.matmul(out=h_ps[:],
                                     lhsT=lhs[:, k * P:(k + 1) * P],
                                     rhs=w1_t[k][:],
                                     start=(k == 0), stop=(k == 3))
                    nc.tensor.matmul(out=a_ps[:],
                                     lhsT=lhs[:, k * P:(k + 1) * P],
                                     rhs=w1a_t[k][:],
                                     start=(k == 0), stop=(k == 3))
                cat = sb.tile([P, 576], fp8, tag="cat")
                nc.scalar.activation(cat[:, :HD], h_ps[:], AF.Copy)
                nc.vector.tensor_copy(cat[:, HD:576].bitcast(f32), a_ps[:])
                nc.scalar.dma_start(ag[t * P:(t + 1) * P, 0:576], cat[:])

            def node_last_t(actt, t):
                if t not in lhs_pend:
                    node_lhs_load(actt, t)
                lhs = lhs_pend.pop(t)
                h_ps = ps.tile([P, TW2], f32, tag="agg", bufs=2)
                for k in range(4):
                    nc.tensor.matmul(out=h_ps[:],
                                     lhsT=lhs[:, k * P:(k + 1) * P],
                                     rhs=w2e_t[k][:],
                                     start=(k == 0), stop=(k == 3))
                cat = sb.tile([P, CO + 2], bf16, tag="cat")
                nc.scalar.activation(cat[:], h_ps[:, 0:CO + 2], AF.Copy)
                nc.scalar.dma_start(ag2[t * P:(t + 1) * P, 0:CO + 2],
                                    cat[:])

            def allgather(ag_in, hcat_f):
                if single:
                    # timeline-sim mode: stand-in copy, no collective
                    nc.sync.dma_start(hcat_f[0:nloc, :], ag_in[:])
                else:
                    nc.gpsimd.collective_compute(
                        "AllGather", ALU.bypass,
                        replica_groups=[list(range(NCORES))],
                        ins=[ag_in[:].opt()],
                        outs=[hcat_f[:, :].opt()],
                    )

            # ---------- edge aggregation (one super == one dst tile)
            def agg_layer(hcat_f, ag_in, tw, hwid, nhh, flush,
                          pre=None, post=None, pcn=POOL_CHUNKS):
                for s in range(bpc):
                    if pre is not None:
                        pre(s)
                    ioff = s * P * G * 8
                    idx_t = sb.tile([P, G * 8], i16, tag="idx")
                    nc.sync.dma_start(
                        idx_t[:],
                        idx_ap[ioff:ioff + P * G * 8]
                        .rearrange("(p x) -> p x", x=G * 8))
                    is8 = (tw == TW)
                    if is8:
                        adf = sb.tile([P, nhh], f32, tag="adf")
                        nc.sync.dma_start(
                            adf[:],
                            ag_in[s * P:(s + 1) * P, 544:576].bitcast(f32))
                        adst_t = sb.tile([P, nhh], bf16, tag="adst")
                        nc.vector.tensor_copy(adst_t[:], adf[:])
                    else:
                        adst_t = sb.tile([P, nhh], bf16, tag="adst")
                        nc.sync.dma_start(
                            adst_t[:],
                            ag_in[s * P:(s + 1) * P,
                                  hwid + nhh:hwid + 2 * nhh])

                    rowg = bb.tile([P, G * tw], fp8 if is8 else bf16,
                                   tag="rowg")
                    rview = rowg[:].rearrange("p (b e) -> p b e", e=tw)
                    nc.gpsimd.dma_gather(
                        out_ap=rview[:, 0:K1, :],
                        in_ap=hcat_f[0:SPL, :],
                        idxs_ap=idx_t[:, 0:K1 * 8],
                        num_idxs=K1 * P, num_idxs_reg=K1 * P,
                        elem_size=tw, elem_step=tw, single_packet=False)
                    nc.gpsimd.dma_gather(
                        out_ap=rview[:, K1:G, :],
                        in_ap=hcat_f[SPL:npad, :],
                        idxs_ap=idx_t[:, K1 * 8:G * 8],
                        num_idxs=K2 * P, num_idxs_reg=K2 * P,
                        elem_size=tw, elem_step=tw, single_packet=False)

                    poff = s * P * 2 * G * P
                    pmpt_t = bb.tile([P, 2 * G * P], fp8, tag="pmpt")
                    nc.sync.dma_start(
                        pmpt_t[:],
                        pmpt_ap[poff:poff + P * 2 * G * P]
                        .rearrange("(p x) -> p x", x=2 * G * P))

                    ptv = pmpt_t[:, G * P:2 * G * P]
                    ade_ps = ps.tile([P, G * nhh], f32, tag="ade", bufs=2)
                    for g in range(G):
                        nc.tensor.matmul(
                            out=ade_ps[:, g * nhh:(g + 1) * nhh],
                            lhsT=ptv[:, g * P:(g + 1) * P],
                            rhs=adst_t[:],
                            start=True, stop=True)

                    ne = G * nhh
                    logit = sb.tile([P, G * NH], bf16, tag="logit")
                    tmp = sb.tile([P, G * NH], bf16, tag="tmp")
                    asrc_v = (rview[:, :, 512:544].bitcast(f32) if is8
                              else rview[:, :, hwid:hwid + nhh])
                    nc.vector.tensor_tensor(
                        out=logit[:, :ne].rearrange("p (g h) -> p g h", g=G),
                        in0=asrc_v,
                        in1=ade_ps[:].rearrange("p (g h) -> p g h", g=G),
                        op=ALU.add)
                    nc.vector.tensor_scalar_mul(tmp[:, :ne], logit[:, :ne],
                                                NEG)
                    nc.vector.tensor_tensor(out=logit[:, :ne],
                                            in0=logit[:, :ne],
                                            in1=tmp[:, :ne], op=ALU.max)
                    ex = sb.tile([P, G * NH], bf16, tag="ex")
                    nc.scalar.activation(ex[:, :ne], logit[:, :ne], AF.Exp)

                    sc = bb.tile([P, G * hwid], bf16, tag="sc")

                    def sc_mult(eng, g0, g1):
                        eng.tensor_tensor(
                            out=sc[:, g0 * hwid:g1 * hwid].rearrange(
                                "p (g h c) -> p g h c", g=g1 - g0, c=HC),
                            in0=rview[:, g0:g1, :hwid].rearrange(
                                "p g (h c) -> p g h c", c=HC),
                            in1=ex[:, g0 * nhh:g1 * nhh].rearrange(
                                "p (g h one) -> p g h one", g=g1 - g0, one=1)
                                .broadcast_to([P, g1 - g0, nhh, HC]),
                            op=ALU.mult)

                    pc = min(pcn, G - 1)
                    sc_mult(nc.vector, 0, G - pc)
                    if pc:
                        sc_mult(nc.gpsimd, G - pc, G)

                    o_ps = ps.tile([P, hwid], f32, tag="agg", bufs=2)
                    d_ps = ps.tile([P, nhh], f32, tag="den", bufs=2)
                    for g in range(G):
                        nc.tensor.matmul(
                            out=o_ps[:],
                            lhsT=pmpt_t[:, g * P:(g + 1) * P],
                            rhs=sc[:, g * hwid:(g + 1) * hwid],
                            start=(g == 0), stop=(g == G - 1))
                        nc.tensor.matmul(
                            out=d_ps[:],
                            lhsT=pmpt_t[:, g * P:(g + 1) * P],
                            rhs=ex[:, g * nhh:(g + 1) * nhh],
                            start=(g == 0), stop=(g == G - 1))
                    flush(s, o_ps, d_ps)
                    if post is not None:
                        post(s)

            def flush_big(t, o_ps, d_ps, b_t, actt_next):
                den = sb.tile([P, NH], f32, tag="den_sb")
                nc.vector.tensor_scalar_add(den[:], d_ps[:], 1e-20)
                rden = sb.tile([P, NH], f32, tag="rden")
                nc.vector.reciprocal(rden[:], den[:])
                o_sb = sb.tile([P, HD], f32, tag="osb")
                nc.vector.tensor_tensor(
                    out=o_sb[:].rearrange("p (h c) -> p h c", c=HC),
                    in0=o_ps[:].rearrange("p (h c) -> p h c", c=HC),
                    in1=rden[:].rearrange("p (h one) -> p h one", one=1)
                        .broadcast_to([P, NH, HC]),
                    op=ALU.mult)
                o_rel = sb.tile([P, HD], bf16, tag="orel")
                nc.vector.tensor_tensor(out=o_rel[:], in0=o_sb[:],
                                        in1=b_t[:], op=ALU.add)
                nc.vector.tensor_scalar_max(o_rel[:], o_rel[:], 0.0)
                tr_ps = ps.tile([P, HD], bf16, tag="tr", bufs=1)
                for k in range(4):
                    nc.tensor.transpose(out=tr_ps[:, k * P:(k + 1) * P],
                                        in_=o_rel[:, k * P:(k + 1) * P],
                                        identity=identb_t[:])
                trsb = sb.tile([P, HD], bf16, tag="trsb")
                nc.scalar.activation(trsb[:], tr_ps[:], AF.Copy)
                nc.sync.dma_start(actt_next[t * P:(t + 1) * P, :],
                                  trsb[:])

            def flush_l2(t, o_ps, d_ps):
                den = sb.tile([P, 1], f32, tag="den_sb")
                nc.vector.tensor_scalar_add(den[:], d_ps[:], 1e-20)
                rden = sb.tile([P, 1], f32, tag="rden")
                nc.vector.reciprocal(rden[:], den[:])
                o_sb = sb.tile([P, CO], f32, tag="osb")
                nc.vector.tensor_tensor(
                    out=o_sb[:], in0=o_ps[:],
                    in1=rden[:].broadcast_to([P, CO]), op=ALU.mult)
                nc.vector.tensor_tensor(out=o_sb[:], in0=o_sb[:], in1=b2_t[:],
                                        op=ALU.add)
                mx = sb.tile([P, 1], f32, tag="mx")
                nc.vector.tensor_reduce(out=mx[:], in_=o_sb[:, :OUTC],
                                        axis=mybir.AxisListType.X, op=ALU.max)
                t2 = sb.tile([P, OUTC], f32, tag="t2")
                nc.vector.tensor_tensor(out=t2[:], in0=o_sb[:, :OUTC],
                                        in1=mx[:].broadcast_to([P, OUTC]),
                                        op=ALU.subtract)
                exl = sb.tile([P, OUTC], f32, tag="exl")
                nc.scalar.activation(exl[:], t2[:], AF.Exp)
                sm = sb.tile([P, 1], f32, tag="sm")
                nc.vector.tensor_reduce(out=sm[:], in_=exl[:],
                                        axis=mybir.AxisListType.X, op=ALU.add)
                ls = sb.tile([P, 1], f32, tag="ls")
                nc.scalar.activation(ls[:], sm[:], AF.Ln)
                res = sb.tile([P, OUTC], f32, tag="res")
                nc.vector.tensor_tensor(out=res[:], in0=t2[:],
                                        in1=ls[:].broadcast_to([P, OUTC]),
                                        op=ALU.subtract)
                nc.sync.dma_start(out_ap[t * P:(t + 1) * P, :], res[:])

            # ---------- the program
            LAG = 3
            node0()
            allgather(ag0, hcat0)
            agg_layer(hcat0, ag0, TW, HD, NH,
                      lambda t, o, d: flush_big(t, o, d, b0_t, actt1),
                      pre=lambda s: (node_lhs_load(actt1, s - LAG + 1)
                                     if s >= LAG - 1 else None),
                      post=lambda s: (node_mid_t(actt1, ag1, s - LAG)
                                      if s >= LAG else None))
            for t in range(bpc - LAG, bpc):
                node_mid_t(actt1, ag1, t)
            allgather(ag1, hcat1)
            agg_layer(hcat1, ag1, TW, HD, NH,
                      lambda t, o, d: flush_big(t, o, d, b1_t, actt2),
                      pre=lambda s: (node_lhs_load(actt2, s - LAG + 1)
                                     if s >= LAG - 1 else None),
                      post=lambda s: (node_last_t(actt2, s - LAG)
                                      if s >= LAG else None))
            for t in range(bpc - LAG, bpc):
                node_last_t(actt2, t)
            allgather(ag2, hcat2)
            agg_layer(hcat2, ag2, TW2, CO, 1, flush_l2)

            if dbg:
                dag0 = nc.dram_tensor("dag0", [nloc, TW], fp8,
                                      kind="ExternalOutput").ap()
                dag1 = nc.dram_tensor("dag1", [nloc, TW], fp8,
                                      kind="ExternalOutput").ap()
                dag2 = nc.dram_tensor("dag2", [nloc, TW2], bf16,
                                      kind="ExternalOutput").ap()
                nc.sync.dma_start(dag0[:, :], ag0[:, :])
                nc.sync.dma_start(dag1[:, :], ag1[:, :])
                nc.sync.dma_start(dag2[:, :], ag2[:, :])

    nc.compile()
    return nc


# ------------------------------------------------------------------ runners

_CACHE = {}


def _get_program(dims):
    key = tuple(sorted(dims.items()))
    if key not in _CACHE:
        _CACHE[key] = _build(dims)
    return _CACHE[key]


def make_in_maps(x, W0, as0, ad0, b0, W1, as1, ad1, b1, W2, as2, ad2, b2,
                 dims, per_core, new_id):
    npad, nloc = dims["npad"], dims["nloc"]
    xp = np.zeros((npad, D0), np.float32)
    xp[new_id] = np.asarray(x, np.float32)

    W0 = np.asarray(W0, np.float32)
    W1 = np.asarray(W1, np.float32)
    W2 = np.asarray(W2, np.float32)
    a0 = _block_diag_a(np.asarray(as0, np.float32),
                       np.asarray(ad0, np.float32))
    a1 = _block_diag_a(np.asarray(as1, np.float32),
                       np.asarray(ad1, np.float32))
    w2e = np.zeros((HD, TW2), np.float32)
    w2e[:, :OUTC] = W2
    w2e[:, CO] = W2 @ np.asarray(as2, np.float32)[0]
    w2e[:, CO + 1] = W2 @ np.asarray(ad2, np.float32)[0]
    b2p = np.zeros((1, CO), np.float32)
    b2p[0, :OUTC] = b2

    shared = {
        "w0h": W0.astype(bfloat16),
        "w0a": (W0 @ a0).astype(bfloat16),
        "w1": W1.astype(bfloat16),
        "w1a": (W1 @ a1).astype(bfloat16),
        "w2e": w2e.astype(bfloat16),
        "b0r": np.asarray(b0, np.float32).reshape(1, HD),
        "b1r": np.asarray(b1, np.float32).reshape(1, HD),
        "b2r": b2p,
        "identb": np.eye(P, dtype=np.float32).astype(bfloat16),
    }
    in_maps = []
    for c in range(NCORES):
        m = dict(shared)
        m["xt"] = np.ascontiguousarray(
            xp[c * nloc:(c + 1) * nloc].T).astype(bfloat16)
        m.update(per_core[c])
        in_maps.append(m)
    return in_maps


def assemble_output(results, dims, new_id):
    n = dims["n"]
    full = np.concatenate([results[c]["out"] for c in range(NCORES)], axis=0)
    return np.ascontiguousarray(full[new_id[:n]])


def kernel(x, edge_index, W0, as0, ad0, b0, W1, as1, ad1, b1,
           W2, as2, ad2, b2):
    from concourse import bass_utils

    n = x.shape[0]
    dims, per_core, new_id = _prep(np.asarray(edge_index), n)
    prog = _get_program(dims)
    in_maps = make_in_maps(x, W0, as0, ad0, b0, W1, as1, ad1, b1,
                           W2, as2, ad2, b2, dims, per_core, new_id)
    res = bass_utils.run_bass_kernel_spmd(prog, in_maps,
                                          core_ids=list(range(NCORES)))
    return assemble_output(res.results, dims, new_id)


# revision 57
# speedup vs baseline: 1.0125x; 1.0125x over previous
"""3-layer GAT on 8 Trainium2 NeuronCores — v3.

Strategy (edge-parallel, dst-sharded).  Revised after TimelineSim
profiling (per-instruction SWDGE overhead on Pool + f32 gather traffic
dominated v1) and hardware probing (multi-index `indirect_dma_start`
offsets silently degrade to one contiguous block per partition, so the
batched gathers use the `dma_gather` ISA op, which was probed to work,
including strided rows and column-sliced tables).

  - Nodes are relabeled into degree-balanced 128-node bins; each core
    owns bpc bins (dst-sharded).  A "super" == one dst tile.
  - Per layer, the node phase computes packed rows
    [h | a_src | a_dst | pad] (bf16, 640 cols for layers 0/1, 128 cols
    for layer 2 -- widths forced by dma_gather's 256B granularity); an
    AllGather replicates them into a [npad, width] table.
  - dma_gather indices are int16, so the table is addressed in two
    halves (rows < 32768 and the rest).  The host orders each tile's
    edges low-half first (chunks 0..K1-1), high-half after, padding
    each half to chunk boundaries; every super then does exactly two
    batched row gathers.
  - Per-edge a_dst comes from a per-chunk one-hot matmul (pt) against
    the dst tile's a_dst rows; a_src rides inside the gathered row.
  - The one-hot scatter (pm) and lookup (pt) matrices are pure graph
    structure: host-built, shipped as fp8e4m3 (0/1 exact; probed OK as
    lhsT against bf16 rhs), one combined load per super.
  - exp(leaky_relu(a_src+a_dst)) on DVE/ACT (bf16), sc = h*alpha on
    DVE (optionally part on Pool), then per chunk one scatter-matmul
    (messages) + one denominator matmul accumulate in PSUM; flush
    normalizes, adds bias, relu, and writes transposed activations
    (PE transpose + one ACT copy + one strided DMA).
  - Pad edges point at row 0 with all-zero one-hot columns.

The module builds and compiles the Bass program on first call (keyed by
input shapes) and reuses it afterwards.
"""

import sys

try:
    import concourse  # noqa: F401  (provided via PYTHONPATH on axon hosts)
except ImportError:
    sys.path.insert(0, "/opt/trn_rl_repo")

import heapq

import numpy as np
from ml_dtypes import bfloat16, float8_e4m3

import concourse.bacc as bacc
import concourse.mybir as mybir
import concourse.tile as tile

P = 128
NCORES = 8
NH = 8          # heads (layers 0/1)
HC = 64         # channels per head
HD = NH * HC    # 512
D0 = 128        # input feature dim
OUTC = 40       # final classes
CO = 64         # padded layer-2 width
TW = 768        # table row bytes, layers 0/1 (fp8 tensor):
                #   h fp8 [0:512] | asrc f32 [512:544] | adst f32 [544:576] | pad
TW2 = 128       # table row width, layer 2:   h2(64)|asrc2(1)|adst2(1)|pad
NEG = 0.2       # leaky relu slope
LO = 32768      # int16 index limit (table split point)

# number of trailing chunks per super whose alpha*h multiply runs on the
# Pool engine instead of DVE (load balancing knob; 0 = all on DVE)
POOL_CHUNKS = 4

f32 = mybir.dt.float32
bf16 = mybir.dt.bfloat16
fp8 = mybir.dt.float8e4
i16 = mybir.dt.int16
AF = mybir.ActivationFunctionType
ALU = mybir.AluOpType


# ----------------------------------------------------------------- host prep

def _balance_bins(deg, nbins):
    """Assign each node to a 128-slot bin, balancing summed in-degree."""
    n = deg.shape[0]
    order = np.argsort(-deg, kind="stable")
    bin_of = np.empty(n, np.int32)
    slot_of = np.empty(n, np.int32)
    counts = np.zeros(nbins, np.int32)
    loads = np.zeros(nbins, np.int64)
    heap = [(0, b) for b in range(nbins)]
    heapq.heapify(heap)
    for node in order:
        while True:
            _, b = heapq.heappop(heap)
            if counts[b] < P:
                break
        bin_of[node] = b
        slot_of[node] = counts[b]
        counts[b] += 1
        loads[b] += deg[node]
        if counts[b] < P:
            heapq.heappush(heap, (int(loads[b]), b))
    return bin_of, slot_of, loads


def _wrap16(flat):
    """Lay a flat int16 index vector in dma_gather's wrapped order:
    position i -> [i % 16, i // 16], replicated across the 8 gpsimd
    core groups (partitions 16..127)."""
    n = flat.shape[0]
    lay = np.zeros((P, n // 16), np.int16)
    lay[:16, :] = flat.reshape(n // 16, 16).T
    for grp in range(1, 8):
        lay[grp * 16:(grp + 1) * 16, :] = lay[:16, :]
    return lay


def _prep(edge_index, n_nodes):
    src = np.asarray(edge_index[0], dtype=np.int64)
    dst = np.asarray(edge_index[1], dtype=np.int64)
    loop = np.arange(n_nodes, dtype=np.int64)
    src = np.concatenate([src, loop])
    dst = np.concatenate([dst, loop])

    deg = np.bincount(dst, minlength=n_nodes)
    bpc = -(-n_nodes // (P * NCORES))          # bins per core
    nbins = bpc * NCORES
    npad = nbins * P
    nloc = bpc * P

    bin_of, slot_of, loads = _balance_bins(deg, nbins)
    new_id = bin_of.astype(np.int64) * P + slot_of

    e_src = new_id[src]
    e_dst = new_id[dst]
    e_bin = e_dst >> 7
    e_slot = e_dst & 127

    order_e = np.argsort(e_bin, kind="stable")
    starts = np.zeros(nbins + 1, np.int64)
    starts[1:] = np.cumsum(np.bincount(e_bin, minlength=nbins))

    # per-bin low/high (by source id) counts -> uniform chunk split.
    # The split point S is searched to minimize total chunks per tile,
    # subject to both table halves having < 32768 rows (int16 indices).
    smin = max(1, npad - LO)
    best = None
    for S in range(smin, LO + 1, 256):
        is_lo = (e_src < S)
        nlo = np.bincount(e_bin[is_lo], minlength=nbins)
        nhi = np.bincount(e_bin[~is_lo], minlength=nbins)
        k1 = int(-(-nlo.max() // P))
        k2 = int(-(-nhi.max() // P))
        if best is None or k1 + k2 < best[0]:
            best = (k1 + k2, k1, S)
    G, K1, SPL = best

    idx_arr = np.zeros((NCORES, bpc, P, G * 8), np.int16)
    pmpt = np.zeros((NCORES, bpc, P, 2 * G * P), float8_e4m3)
    one8 = float8_e4m3(1.0)

    for b in range(nbins):
        c, t = divmod(b, bpc)
        es = order_e[starts[b]:starts[b + 1]]
        srcs = e_src[es]
        slots = e_slot[es].astype(np.int64)
        lo_m = srcs < SPL
        for half, (mask, base, k0, kcnt) in enumerate(
                [(lo_m, 0, 0, K1), (~lo_m, SPL, K1, G - K1)]):
            sv = (srcs[mask] - base).astype(np.int16)
            sl = slots[mask]
            ncap = kcnt * P
            flat = np.zeros(ncap, np.int16)
            flat[:sv.shape[0]] = sv
            idx_arr[c, t, :, k0 * 8:(k0 + kcnt) * 8] = _wrap16(flat)
            i = np.arange(sv.shape[0])
            p_of = i & 127
            g_of = k0 + (i >> 7)
            pmpt[c, t, p_of, g_of * P + sl] = one8                 # pm
            pmpt[c, t, sl, G * P + g_of * P + p_of] = one8         # pt

    per_core = []
    for c in range(NCORES):
        per_core.append({
            "idx": np.ascontiguousarray(idx_arr[c]).reshape(-1),
            "pmpt": np.ascontiguousarray(pmpt[c]).reshape(-1),
        })

    dims = dict(n=n_nodes, bpc=bpc, nbins=nbins, npad=npad, nloc=nloc,
                G=G, K1=K1, SPL=SPL)
    return dims, per_core, new_id


def _block_diag_a(att_s, att_d):
    """[NH,HC]x2 -> [HD, 2*NH] block matrix for a = h @ A."""
    a = np.zeros((HD, 2 * NH), np.float32)
    r = np.arange(HD)
    h = r >> 6
    c = r & 63
    a[r, h] = att_s[h, c]
    a[r, NH + h] = att_d[h, c]
    return a


# ------------------------------------------------------------- device build

def _build(dims, single=False, dbg=False):
    npad, nloc, bpc = dims["npad"], dims["nloc"], dims["bpc"]
    G, K1, SPL = dims["G"], dims["K1"], dims["SPL"]
    K2 = G - K1

    nc = bacc.Bacc("TRN2", target_bir_lowering=False, debug=False,
                   enable_asserts=True,
                   num_devices=1 if single else NCORES)

    # inputs
    xt_ap = nc.dram_tensor("xt", [D0, nloc], bf16, kind="ExternalInput").ap()
    idx_ap = nc.dram_tensor("idx", [bpc * P * G * 8], i16,
                            kind="ExternalInput").ap()
    pmpt_ap = nc.dram_tensor("pmpt", [bpc * P * 2 * G * P], fp8,
                             kind="ExternalInput").ap()
    w0h_ap = nc.dram_tensor("w0h", [D0, HD], bf16, kind="ExternalInput").ap()
    w0a_ap = nc.dram_tensor("w0a", [D0, 2 * NH], bf16,
                            kind="ExternalInput").ap()
    w1_ap = nc.dram_tensor("w1", [HD, HD], bf16, kind="ExternalInput").ap()
    w1a_ap = nc.dram_tensor("w1a", [HD, 2 * NH], bf16,
                            kind="ExternalInput").ap()
    w2e_ap = nc.dram_tensor("w2e", [HD, TW2], bf16,
                            kind="ExternalInput").ap()
    b0_ap = nc.dram_tensor("b0r", [1, HD], f32, kind="ExternalInput").ap()
    b1_ap = nc.dram_tensor("b1r", [1, HD], f32, kind="ExternalInput").ap()
    b2_ap = nc.dram_tensor("b2r", [1, CO], f32, kind="ExternalInput").ap()
    idb_ap = nc.dram_tensor("identb", [P, P], bf16, kind="ExternalInput").ap()
    out_ap = nc.dram_tensor("out", [nloc, OUTC], f32, kind="ExternalOutput").ap()

    with tile.TileContext(nc) as tc:
        with tc.tile_pool(name="const", bufs=1) as cp, \
             tc.tile_pool(name="work", bufs=3) as sb, \
             tc.tile_pool(name="big", bufs=2) as bb, \
             tc.tile_pool(name="psum", bufs=2, space="PSUM") as ps, \
             tc.tile_pool(name="dram", bufs=1, space="DRAM") as dp:

            # ---------- persistent constants in SBUF
            identb_t = cp.tile([P, P], bf16)
            nc.sync.dma_start(identb_t[:], idb_ap[:])
            ones_t = cp.tile([1, P], f32)
            nc.gpsimd.memset(ones_t[:], 1.0)

            w0h_t = cp.tile([P, HD], bf16)
            nc.sync.dma_start(w0h_t[:], w0h_ap[:])
            w0a_t = cp.tile([P, 2 * NH], bf16)
            nc.sync.dma_start(w0a_t[:], w0a_ap[:])
            w1_t = [cp.tile([P, HD], bf16, name=f"w1c{k}", tag=f"w1_{k}")
                    for k in range(4)]
            w1a_t = [cp.tile([P, 2 * NH], bf16, name=f"w1ac{k}", tag=f"w1a_{k}")
                     for k in range(4)]
            w2e_t = [cp.tile([P, TW2], bf16, name=f"w2ec{k}", tag=f"w2e_{k}")
                     for k in range(4)]
            for k in range(4):
                nc.sync.dma_start(w1_t[k][:], w1_ap[k * P:(k + 1) * P, :])
                nc.sync.dma_start(w1a_t[k][:], w1a_ap[k * P:(k + 1) * P, :])
                nc.sync.dma_start(w2e_t[k][:], w2e_ap[k * P:(k + 1) * P, :])

            def bias_tile(b_ap, width, tag):
                row = sb.tile([1, width], f32, tag="brow")
                nc.sync.dma_start(row[:], b_ap[:])
                bps = ps.tile([P, width], f32, tag="agg", bufs=2)
                nc.tensor.matmul(out=bps[:], lhsT=ones_t[:], rhs=row[:],
                                 start=True, stop=True)
                bt = cp.tile([P, width], f32, name=tag, tag=tag)
                nc.scalar.activation(bt[:], bps[:], AF.Copy)
                return bt

            b0_t = bias_tile(b0_ap, HD, "b0t")
            b1_t = bias_tile(b1_ap, HD, "b1t")
            b2_t = bias_tile(b2_ap, CO, "b2t")

            # ---------- DRAM scratch
            ag0 = dp.tile([nloc, TW], fp8)
            ag1 = dp.tile([nloc, TW], fp8)
            ag2 = dp.tile([nloc, TW2], bf16)
            hcat0 = dp.tile([npad, TW], fp8)
            hcat1 = dp.tile([npad, TW], fp8)
            hcat2 = dp.tile([npad, TW2], bf16)
            actt1 = dp.tile([nloc, HD], bf16)
            actt2 = dp.tile([nloc, HD], bf16)

            # ---------- node phases
            def node0():
                for t in range(bpc):
                    xt_t = sb.tile([P, P], bf16, tag="lhs")
                    nc.scalar.dma_start(xt_t[:], xt_ap[:, t * P:(t + 1) * P])
                    h_ps = ps.tile([P, HD], f32, tag="agg", bufs=2)
                    a_ps = ps.tile([P, 2 * NH], f32, tag="den", bufs=2)
                    nc.tensor.matmul(out=h_ps[:], lhsT=xt_t[:], rhs=w0h_t[:],
                                     start=True, stop=True)
                    nc.tensor.matmul(out=a_ps[:], lhsT=xt_t[:], rhs=w0a_t[:],
                                     start=True, stop=True)
                    cat = sb.tile([P, 576], fp8, tag="cat")
                    nc.scalar.activation(cat[:, :HD], h_ps[:], AF.Copy)
                    nc.vector.tensor_copy(cat[:, HD:576].bitcast(f32),
                                          a_ps[:])
                    nc.scalar.dma_start(ag0[t * P:(t + 1) * P, 0:576],
                                        cat[:])

            lhs_pend = {}

            def node_lhs_load(actt, t):
                lhs = sb.tile([P, 4 * P], bf16, tag="lhs")
                nc.scalar.dma_start(lhs[:], actt[t * P:(t + 1) * P, :])
                lhs_pend[t] = lhs

            def node_mid_t(actt, ag, t):
                if t not in lhs_pend:
                    node_lhs_load(actt, t)
                lhs = lhs_pend.pop(t)
                h_ps = ps.tile([P, HD], f32, tag="agg", bufs=2)
                a_ps = ps.tile([P, 2 * NH], f32, tag="den", bufs=2)
                for k in range(4):
                    nc.tensor.matmul(out=h_ps[:],
                                     lhsT=lhs[:, k * P:(k + 1) * P],
                                     rhs=w1_t[k][:],
                                     start=(k == 0), stop=(k == 3))
                    nc.tensor.matmul(out=a_ps[:],
                                     lhsT=lhs[:, k * P:(k + 1) * P],
                                     rhs=w1a_t[k][:],
                                     start=(k == 0), stop=(k == 3))
                cat = sb.tile([P, 576], fp8, tag="cat")
                nc.scalar.activation(cat[:, :HD], h_ps[:], AF.Copy)
                nc.vector.tensor_copy(cat[:, HD:576].bitcast(f32), a_ps[:])
                nc.scalar.dma_start(ag[t * P:(t + 1) * P, 0:576], cat[:])

            def node_last_t(actt, t):
                if t not in lhs_pend:
                    node_lhs_load(actt, t)
                lhs = lhs_pend.pop(t)
                h_ps = ps.tile([P, TW2], f32, tag="agg", bufs=2)
                for k in range(4):
                    nc.tensor.matmul(out=h_ps[:],
                                     lhsT=lhs[:, k * P:(k + 1) * P],
                                     rhs=w2e_t[k][:],
                                     start=(k == 0), stop=(k == 3))
                cat = sb.tile([P, CO + 2], bf16, tag="cat")
                nc.scalar.activation(cat[:], h_ps[:, 0:CO + 2], AF.Copy)
                nc.scalar.dma_start(ag2[t * P:(t + 1) * P, 0:CO + 2],
                                    cat[:])

            def allgather(ag_in, hcat_f):
                if single:
                    # timeline-sim mode: stand-in copy, no collective
                    nc.sync.dma_start(hcat_f[0:nloc, :], ag_in[:])
                else:
                    nc.gpsimd.collective_compute(
                        "AllGather", ALU.bypass,
                        replica_groups=[list(range(NCORES))],
                        ins=[ag_in[:].opt()],
                        outs=[hcat_f[:, :].opt()],
                    )

            # ---------- edge aggregation (one super == one dst tile)
            def agg_layer(hcat_f, ag_in, tw, hwid, nhh, flush,
                          pre=None, post=None, pcn=POOL_CHUNKS):
                for s in range(bpc):
                    if pre is not None:
                        pre(s)
                    ioff = s * P * G * 8
                    idx_t = sb.tile([P, G * 8], i16, tag="idx")
                    nc.sync.dma_start(
                        idx_t[:],
                        idx_ap[ioff:ioff + P * G * 8]
                        .rearrange("(p x) -> p x", x=G * 8))
                    is8 = (tw == TW)
                    if is8:
                        adf = sb.tile([P, nhh], f32, tag="adf")
                        nc.sync.dma_start(
                            adf[:],
                            ag_in[s * P:(s + 1) * P, 544:576].bitcast(f32))
                        adst_t = sb.tile([P, nhh], bf16, tag="adst")
                        nc.vector.tensor_copy(adst_t[:], adf[:])
                    else:
                        adst_t = sb.tile([P, nhh], bf16, tag="adst")
                        nc.sync.dma_start(
                            adst_t[:],
                            ag_in[s * P:(s + 1) * P,
                                  hwid + nhh:hwid + 2 * nhh])

                    rowg = bb.tile([P, G * tw], fp8 if is8 else bf16,
                                   tag="rowg")
                    rview = rowg[:].rearrange("p (b e) -> p b e", e=tw)
                    nc.gpsimd.dma_gather(
                        out_ap=rview[:, 0:K1, :],
                        in_ap=hcat_f[0:SPL, :],
                        idxs_ap=idx_t[:, 0:K1 * 8],
                        num_idxs=K1 * P, num_idxs_reg=K1 * P,
                        elem_size=tw, elem_step=tw, single_packet=False)
                    nc.gpsimd.dma_gather(
                        out_ap=rview[:, K1:G, :],
                        in_ap=hcat_f[SPL:npad, :],
                        idxs_ap=idx_t[:, K1 * 8:G * 8],
                        num_idxs=K2 * P, num_idxs_reg=K2 * P,
                        elem_size=tw, elem_step=tw, single_packet=False)

                    poff = s * P * 2 * G * P
                    pmpt_t = bb.tile([P, 2 * G * P], fp8, tag="pmpt")
                    nc.sync.dma_start(
                        pmpt_t[:],
                        pmpt_ap[poff:poff + P * 2 * G * P]
                        .rearrange("(p x) -> p x", x=2 * G * P))

                    ptv = pmpt_t[:, G * P:2 * G * P]
                    ade_ps = ps.tile([P, G * nhh], f32, tag="ade", bufs=2)
                    for g in range(G):
                        nc.tensor.matmul(
                            out=ade_ps[:, g * nhh:(g + 1) * nhh],
                            lhsT=ptv[:, g * P:(g + 1) * P],
                            rhs=adst_t[:],
                            start=True, stop=True)

                    ne = G * nhh
                    logit = sb.tile([P, G * NH], bf16, tag="logit")
                    tmp = sb.tile([P, G * NH], bf16, tag="tmp")
                    asrc_v = (rview[:, :, 512:544].bitcast(f32) if is8
                              else rview[:, :, hwid:hwid + nhh])
                    nc.vector.tensor_tensor(
                        out=logit[:, :ne].rearrange("p (g h) -> p g h", g=G),
                        in0=asrc_v,
                        in1=ade_ps[:].rearrange("p (g h) -> p g h", g=G),
                        op=ALU.add)
                    nc.vector.tensor_scalar_mul(tmp[:, :ne], logit[:, :ne],
                                                NEG)
                    nc.vector.tensor_tensor(out=logit[:, :ne],
                                            in0=logit[:, :ne],
                                            in1=tmp[:, :ne], op=ALU.max)
                    ex = sb.tile([P, G * NH], bf16, tag="ex")
                    nc.scalar.activation(ex[:, :ne], logit[:, :ne], AF.Exp)

                    sc = bb.tile([P, G * hwid], bf16, tag="sc")

                    def sc_mult(eng, g0, g1):
                        eng.tensor_tensor(
                            out=sc[:, g0 * hwid:g1 * hwid].rearrange(
                                "p (g h c) -> p g h c", g=g1 - g0, c=HC),
                            in0=rview[:, g0:g1, :hwid].rearrange(
                                "p g (h c) -> p g h c", c=HC),
                            in1=ex[:, g0 * nhh:g1 * nhh].rearrange(
                                "p (g h one) -> p g h one", g=g1 - g0, one=1)
                                .broadcast_to([P, g1 - g0, nhh, HC]),
                            op=ALU.mult)

                    pc = min(pcn, G - 1)
                    sc_mult(nc.vector, 0, G - pc)
                    if pc:
                        sc_mult(nc.gpsimd, G - pc, G)

                    o_ps = ps.tile([P, hwid], f32, tag="agg", bufs=2)
                    d_ps = ps.tile([P, nhh], f32, tag="den", bufs=2)
                    for g in range(G):
                        nc.tensor.matmul(
                            out=o_ps[:],
                            lhsT=pmpt_t[:, g * P:(g + 1) * P],
                            rhs=sc[:, g * hwid:(g + 1) * hwid],
                            start=(g == 0), stop=(g == G - 1))
                        nc.tensor.matmul(
                            out=d_ps[:],
                            lhsT=pmpt_t[:, g * P:(g + 1) * P],
                            rhs=ex[:, g * nhh:(g + 1) * nhh],
                            start=(g == 0), stop=(g == G - 1))
                    flush(s, o_ps, d_ps)
                    if post is not None:
                        post(s)

            def flush_big(t, o_ps, d_ps, b_t, actt_next):
                den = sb.tile([P, NH], f32, tag="den_sb")
                nc.vector.tensor_scalar_add(den[:], d_ps[:], 1e-20)
                rden = sb.tile([P, NH], f32, tag="rden")
                nc.vector.reciprocal(rden[:], den[:])
                o_sb = sb.tile([P, HD], f32, tag="osb")
                nc.vector.tensor_tensor(
                    out=o_sb[:].rearrange("p (h c) -> p h c", c=HC),
                    in0=o_ps[:].rearrange("p (h c) -> p h c", c=HC),
                    in1=rden[:].rearrange("p (h one) -> p h one", one=1)
                        .broadcast_to([P, NH, HC]),
                    op=ALU.mult)
                o_rel = sb.tile([P, HD], bf16, tag="orel")
                nc.vector.tensor_tensor(out=o_rel[:], in0=o_sb[:],
                                        in1=b_t[:], op=ALU.add)
                nc.vector.tensor_scalar_max(o_rel[:], o_rel[:], 0.0)
                tr_ps = ps.tile([P, HD], bf16, tag="tr", bufs=1)
                for k in range(4):
                    nc.tensor.transpose(out=tr_ps[:, k * P:(k + 1) * P],
                                        in_=o_rel[:, k * P:(k + 1) * P],
                                        identity=identb_t[:])
                trsb = sb.tile([P, HD], bf16, tag="trsb")
                nc.scalar.activation(trsb[:], tr_ps[:], AF.Copy)
                nc.sync.dma_start(actt_next[t * P:(t + 1) * P, :],
                                  trsb[:])

            def flush_l2(t, o_ps, d_ps):
                den = sb.tile([P, 1], f32, tag="den_sb")
                nc.vector.tensor_scalar_add(den[:], d_ps[:], 1e-20)
                rden = sb.tile([P, 1], f32, tag="rden")
                nc.vector.reciprocal(rden[:], den[:])
                o_sb = sb.tile([P, CO], f32, tag="osb")
                nc.vector.tensor_tensor(
                    out=o_sb[:], in0=o_ps[:],
                    in1=rden[:].broadcast_to([P, CO]), op=ALU.mult)
                nc.vector.tensor_tensor(out=o_sb[:], in0=o_sb[:], in1=b2_t[:],
                                        op=ALU.add)
                mx = sb.tile([P, 1], f32, tag="mx")
                nc.vector.tensor_reduce(out=mx[:], in_=o_sb[:, :OUTC],
                                        axis=mybir.AxisListType.X, op=ALU.max)
                t2 = sb.tile([P, OUTC], f32, tag="t2")
                nc.vector.tensor_tensor(out=t2[:], in0=o_sb[:, :OUTC],
                                        in1=mx[:].broadcast_to([P, OUTC]),
                                        op=ALU.subtract)
                exl = sb.tile([P, OUTC], f32, tag="exl")
                nc.scalar.activation(exl[:], t2[:], AF.Exp)
                sm = sb.tile([P, 1], f32, tag="sm")
                nc.vector.tensor_reduce(out=sm[:], in_=exl[:],
                                        axis=mybir.AxisListType.X, op=ALU.add)
                ls = sb.tile([P, 1], f32, tag="ls")
                nc.scalar.activation(ls[:], sm[:], AF.Ln)
                res = sb.tile([P, OUTC], f32, tag="res")
                nc.vector.tensor_tensor(out=res[:], in0=t2[:],
                                        in1=ls[:].broadcast_to([P, OUTC]),
                                        op=ALU.subtract)
                nc.sync.dma_start(out_ap[t * P:(t + 1) * P, :], res[:])

            # ---------- the program
            LAG = 3
            node0()
            allgather(ag0, hcat0)
            agg_layer(hcat0, ag0, TW, HD, NH,
                      lambda t, o, d: flush_big(t, o, d, b0_t, actt1),
                      pre=lambda s: (node_lhs_load(actt1, s - LAG + 1)
                                     if s >= LAG - 1 else None),
                      post=lambda s: (node_mid_t(actt1, ag1, s - LAG)
                                      if s >= LAG else None))
            for t in range(bpc - LAG, bpc):
                node_mid_t(actt1, ag1, t)
            allgather(ag1, hcat1)
            agg_layer(hcat1, ag1, TW, HD, NH,
                      lambda t, o, d: flush_big(t, o, d, b1_t, actt2),
                      pre=lambda s: (node_lhs_load(actt2, s - LAG + 1)
                                     if s >= LAG - 1 else None),
                      post=lambda s: (node_last_t(actt2, s - LAG)
                                      if s >= LAG else None))
            for t in range(bpc - LAG, bpc):
                node_last_t(actt2, t)
            allgather(ag2, hcat2)
            agg_layer(hcat2, ag2, TW2, CO, 1, flush_l2)

            if dbg:
                dag0 = nc.dram_tensor("dag0", [nloc, TW], fp8,
                                      kind="ExternalOutput").ap()
                dag1 = nc.dram_tensor("dag1", [nloc, TW], fp8,
                                      kind="ExternalOutput").ap()
                dag2 = nc.dram_tensor("dag2", [nloc, TW2], bf16,
                                      kind="ExternalOutput").ap()
                nc.sync.dma_start(dag0[:, :], ag0[:, :])
                nc.sync.dma_start(dag1[:, :], ag1[:, :])
                nc.sync.dma_start(dag2[:, :], ag2[:, :])

    nc.compile()
    return nc


# ------------------------------------------------------------------ runners

_CACHE = {}


def _get_program(dims):
    key = tuple(sorted(dims.items()))
    if key not in _CACHE:
        _CACHE[key] = _build(dims)
    return _CACHE[key]


def make_in_maps(x, W0, as0, ad0, b0, W1, as1, ad1, b1, W2, as2, ad2, b2,
                 dims, per_core, new_id):
    npad, nloc = dims["npad"], dims["nloc"]
    xp = np.zeros((npad, D0), np.float32)
    xp[new_id] = np.asarray(x, np.float32)

    W0 = np.asarray(W0, np.float32)
    W1 = np.asarray(W1, np.float32)
    W2 = np.asarray(W2, np.float32)
    a0 = _block_diag_a(np.asarray(as0, np.float32),
                       np.asarray(ad0, np.float32))
    a1 = _block_diag_a(np.asarray(as1, np.float32),
                       np.asarray(ad1, np.float32))
    w2e = np.zeros((HD, TW2), np.float32)
    w2e[:, :OUTC] = W2
    w2e[:, CO] = W2 @ np.asarray(as2, np.float32)[0]
    w2e[:, CO + 1] = W2 @ np.asarray(ad2, np.float32)[0]
    b2p = np.zeros((1, CO), np.float32)
    b2p[0, :OUTC] = b2

    shared = {
        "w0h": W0.astype(bfloat16),
        "w0a": (W0 @ a0).astype(bfloat16),
        "w1": W1.astype(bfloat16),
        "w1a": (W1 @ a1).astype(bfloat16),
        "w2e": w2e.astype(bfloat16),
        "b0r": np.asarray(b0, np.float32).reshape(1, HD),
        "b1r": np.asarray(b1, np.float32).reshape(1, HD),
        "b2r": b2p,
        "identb": np.eye(P, dtype=np.float32).astype(bfloat16),
    }
    in_maps = []
    for c in range(NCORES):
        m = dict(shared)
        m["xt"] = np.ascontiguousarray(
            xp[c * nloc:(c + 1) * nloc].T).astype(bfloat16)
        m.update(per_core[c])
        in_maps.append(m)
    return in_maps


def assemble_output(results, dims, new_id):
    n = dims["n"]
    full = np.concatenate([results[c]["out"] for c in range(NCORES)], axis=0)
    return np.ascontiguousarray(full[new_id[:n]])


def kernel(x, edge_index, W0, as0, ad0, b0, W1, as1, ad1, b1,
           W2, as2, ad2, b2):
    from concourse import bass_utils

    n = x.shape[0]
    dims, per_core, new_id = _prep(np.asarray(edge_index), n)
    prog = _get_program(dims)
    in_maps = make_in_maps(x, W0, as0, ad0, b0, W1, as1, ad1, b1,
                           W2, as2, ad2, b2, dims, per_core, new_id)
    res = bass_utils.run_bass_kernel_spmd(prog, in_maps,
                                          core_ids=list(range(NCORES)))
    return assemble_output(res.results, dims, new_id)
